# revision 22
# baseline (speedup 1.0000x reference)
"""Column-wise RMS normalization on 8 Trainium2 NeuronCores.

Computes y = x * rsqrt(sum(x*x, axis=0) + eps) for x [32768, 2048] f32.

Sharding: column-parallel — each core owns a contiguous block of 256
columns, making the per-column sum-of-squares entirely core-local (no
collectives). Within a core the shard is viewed as [128 p, 256 t, 256 c]
(row = p*256 + t) so every DMA moves >=8KB contiguous runs per partition.

Single-read strategy: the f32 shard is DMA'd from HBM exactly once,
cast to fp16 on the fly (SWDGE cast DMA) into a persistent SBUF cache
(16MB/core). Pass A squares the cache (DVE) and reduces over partitions
with TensorE ones-matmuls into PSUM; the scale rsqrt(u+eps) is computed
via ACT Sqrt + DVE fast reciprocal and broadcast to all partitions with
a K=1 matmul. Pass B multiplies the cached fp16 x by the broadcast scale
(DVE) and DMAs f32 results out. HBM traffic = 32MB in + 32MB out per
core, the roofline floor.
"""

import numpy as np

import concourse.bacc as bacc
import concourse.bass as bass
import concourse.tile as tile
from concourse import mybir
from concourse.bass_utils import run_bass_kernel_spmd

N, D = 32768, 2048
EPS = 1e-6
NCORES = 8
C = D // NCORES  # 256 columns per core
P = 128          # partitions
T = N // P       # 256 rows per partition
G = 8            # row-group (t) per DMA / compute chunk
NG = T // G      # 32 groups

_NC = None


def _build() -> bass.Bass:
    nc = bacc.Bacc("TRN2", target_bir_lowering=False, enable_partition_id=False)
    x = nc.dram_tensor("x", [N, C], mybir.dt.float32, kind="ExternalInput")
    y = nc.dram_tensor("y", [N, C], mybir.dt.float32, kind="ExternalOutput")
    xv = x[:, :].rearrange("(p t) c -> p t c", p=P)
    yv = y[:, :].rearrange("(p t) c -> p t c", p=P)

    with tile.TileContext(nc) as tc:
        with (
            tc.tile_pool(name="cache", bufs=1) as cachep,
            tc.tile_pool(name="consts", bufs=1) as consts,
            tc.tile_pool(name="sq", bufs=2) as sqp,
            tc.tile_pool(name="outs", bufs=4) as outp,
            tc.tile_pool(name="scale", bufs=1) as scalep,
            tc.tile_pool(name="ps", bufs=1, space="PSUM") as psp,
        ):
            xc = cachep.tile([P, T, C], mybir.dt.float16)
            ones_col = consts.tile([P, 1], mybir.dt.float16)
            nc.vector.memset(ones_col, 1.0)
            ones_row = consts.tile([1, P], mybir.dt.float32)
            nc.vector.memset(ones_row, 1.0)
            eps_t = consts.tile([P, 1], mybir.dt.float32)
            nc.vector.memset(eps_t, EPS)

            # u_ps holds 2 interleaved partial column-sum vectors (even/odd t)
            u_ps = psp.tile([1, 2 * C], mybir.dt.float32)
            s_ps = psp.tile([P, 1, C], mybir.dt.float32)

            # Pass A: cast-DMA f32->fp16 into the persistent cache (SWDGE),
            # square on DVE, reduce over partitions (PE ones-matmul
            # accumulate into PSUM). 2MB cast-DMAs for the bulk, one 1MB
            # group, then a short G=2 tail so the final square->matmul
            # chain into the scale computation is short.
            GI = 2 * G
            in_groups = (
                [(j * GI, GI) for j in range(T // GI - 1)]
                + [(T - GI, G)]
                + [(T - G + 2 * h, 2) for h in range(G // 2)]
            )
            nmm = T // 2
            k = 0
            for t0, g in in_groups:
                ts_ = slice(t0, t0 + g)
                nc.gpsimd.dma_start(out=xc[:, ts_, :], in_=xv[:, ts_, :])
                sq = sqp.tile([P, g, C], mybir.dt.float16, tag="sq")
                nc.vector.tensor_mul(sq, xc[:, ts_, :], xc[:, ts_, :])
                for h in range(g // 2):
                    rhs = sq[:, 2 * h : 2 * h + 2, :].rearrange("p t c -> p (t c)")
                    nc.tensor.matmul(
                        u_ps[:, :],
                        lhsT=ones_col[:, :],
                        rhs=rhs,
                        start=(k == 0),
                        stop=(k == nmm - 1),
                    )
                    k += 1

            # Scale: u = even+odd partials; s = 1/sqrt(u+eps) computed on the
            # narrow [1, C] vector, THEN broadcast to all partitions with a
            # K=1 matmul into PSUM. Pass-B muls read s straight from PSUM,
            # which keeps the post-broadcast hop off the critical path.
            u_sb = scalep.tile([1, C], mybir.dt.float32)
            upair = u_ps[:, :].rearrange("p (t c) -> p c t", t=2)
            nc.vector.reduce_sum(u_sb, upair, axis=mybir.AxisListType.X)
            tsq = scalep.tile([1, C], mybir.dt.float32)
            nc.scalar.activation(
                out=tsq[:, :],
                in_=u_sb[:, :],
                func=mybir.ActivationFunctionType.Sqrt,
                bias=eps_t[0:1, :],
                scale=1.0,
            )
            s1 = scalep.tile([1, C], mybir.dt.float32)
            nc.vector.reciprocal_approx_fast(out=s1[:, :], in_=tsq[:, :])
            nc.tensor.matmul(
                s_ps[:, 0, :], lhsT=ones_row[:, :], rhs=s1[:, :], start=True, stop=True
            )

            # Pass B: scale cached x, write out. First 8 rows go in small
            # G=2 chunks so the first out-DMA launches as early as possible
            # after the scale is ready.
            out_groups = [(2 * h, 2) for h in range(G // 2)] + [
                (G + j * G, G) for j in range(NG - 1)
            ]
            for t0, g in out_groups:
                ts_ = slice(t0, t0 + g)
                ot = outp.tile([P, g, C], mybir.dt.float32, tag="ot")
                nc.vector.tensor_mul(
                    ot, xc[:, ts_, :], s_ps[:, :, :].to_broadcast((P, g, C))
                )
                nc.sync.dma_start(out=yv[:, ts_, :], in_=ot)
    nc.compile()
    return nc


def _get_nc() -> bass.Bass:
    global _NC
    if _NC is None:
        _NC = _build()
    return _NC


def kernel(x) -> np.ndarray:
    x = np.asarray(x, dtype=np.float32)
    assert x.shape == (N, D), x.shape
    nc = _get_nc()
    in_maps = [
        {"x": np.ascontiguousarray(x[:, i * C : (i + 1) * C])} for i in range(NCORES)
    ]
    try:
        res = run_bass_kernel_spmd(nc, in_maps, core_ids=list(range(NCORES)))
    except Exception:
        # Transient NRT/device hiccups (e.g. a previous process's profiling
        # session left a core wedged) recover after a short pause.
        import time

        time.sleep(5)
        res = run_bass_kernel_spmd(nc, in_maps, core_ids=list(range(NCORES)))
    return np.concatenate([r["y"] for r in res.results], axis=1)


# revision 24
# speedup vs baseline: 1.0098x; 1.0098x over previous
"""Column-wise RMS normalization on 8 Trainium2 NeuronCores.

Computes y = x * rsqrt(sum(x*x, axis=0) + eps) for x [32768, 2048] f32.

Sharding: column-parallel — each core owns a contiguous block of 256
columns, making the per-column sum-of-squares entirely core-local (no
collectives). Within a core the shard is viewed as [128 p, 256 t, 256 c]
(row = p*256 + t) so every DMA moves >=8KB contiguous runs per partition.

Single-read strategy: the f32 shard is DMA'd from HBM exactly once,
cast to fp16 on the fly (SWDGE cast DMA) into a persistent SBUF cache
(16MB/core). Pass A squares the cache (DVE) and reduces over partitions
with TensorE ones-matmuls into PSUM; the scale rsqrt(u+eps) is computed
via ACT Sqrt + DVE fast reciprocal and broadcast to all partitions with
a K=1 matmul. Pass B multiplies the cached fp16 x by the broadcast scale
(DVE) and DMAs f32 results out. HBM traffic = 32MB in + 32MB out per
core, the roofline floor.
"""

import numpy as np

import concourse.bacc as bacc
import concourse.bass as bass
import concourse.tile as tile
from concourse import mybir
from concourse.bass_utils import run_bass_kernel_spmd

N, D = 32768, 2048
EPS = 1e-6
NCORES = 8
C = D // NCORES  # 256 columns per core
P = 128          # partitions
T = N // P       # 256 rows per partition
G = 8            # row-group (t) per DMA / compute chunk
NG = T // G      # 32 groups

_NC = None


def _build() -> bass.Bass:
    nc = bacc.Bacc("TRN2", target_bir_lowering=False, enable_partition_id=False)
    x = nc.dram_tensor("x", [N, C], mybir.dt.float32, kind="ExternalInput")
    y = nc.dram_tensor("y", [N, C], mybir.dt.float32, kind="ExternalOutput")
    xv = x[:, :].rearrange("(p t) c -> p t c", p=P)
    yv = y[:, :].rearrange("(p t) c -> p t c", p=P)

    with tile.TileContext(nc) as tc:
        with (
            tc.tile_pool(name="cache", bufs=1) as cachep,
            tc.tile_pool(name="consts", bufs=1) as consts,
            tc.tile_pool(name="sq", bufs=2) as sqp,
            tc.tile_pool(name="outs", bufs=4) as outp,
            tc.tile_pool(name="scale", bufs=1) as scalep,
            tc.tile_pool(name="ps", bufs=1, space="PSUM") as psp,
        ):
            xc = cachep.tile([P, T, C], mybir.dt.float16)
            ones_col = consts.tile([P, 1], mybir.dt.float16)
            nc.vector.memset(ones_col, 1.0)
            ones_row = consts.tile([1, P], mybir.dt.float32)
            nc.vector.memset(ones_row, 1.0)
            eps_t = consts.tile([P, 1], mybir.dt.float32)
            nc.vector.memset(eps_t, EPS)

            # u_ps holds 2 interleaved partial column-sum vectors (even/odd t)
            u_ps = psp.tile([1, 2 * C], mybir.dt.float32)
            s_ps = psp.tile([P, 1, C], mybir.dt.float32)

            # Pass A: cast-DMA f32->fp16 into the persistent cache (SWDGE),
            # square on DVE, reduce over partitions (PE ones-matmul
            # accumulate into PSUM). 2MB cast-DMAs for the bulk, one 1MB
            # group, then a short G=2 tail so the final square->matmul
            # chain into the scale computation is short.
            GI = 2 * G
            in_groups = (
                [(j * GI, GI) for j in range(T // GI - 1)]
                + [(T - GI, G)]
                + [(T - G + 2 * h, 2) for h in range(G // 2)]
            )
            nmm = T // 2
            k = 0
            for t0, g in in_groups:
                ts_ = slice(t0, t0 + g)
                nc.gpsimd.dma_start(out=xc[:, ts_, :], in_=xv[:, ts_, :])
                # Tail (g==2) squares get their own 4-deep slot set so they
                # don't stall on PE consuming the big groups' sq slots.
                if g > 2:
                    sq = sqp.tile([P, g, C], mybir.dt.float16, tag="sq", bufs=2)
                else:
                    sq = sqp.tile([P, g, C], mybir.dt.float16, tag="sqt", bufs=4)
                nc.vector.tensor_mul(sq, xc[:, ts_, :], xc[:, ts_, :])
                for h in range(g // 2):
                    rhs = sq[:, 2 * h : 2 * h + 2, :].rearrange("p t c -> p (t c)")
                    nc.tensor.matmul(
                        u_ps[:, :],
                        lhsT=ones_col[:, :],
                        rhs=rhs,
                        start=(k == 0),
                        stop=(k == nmm - 1),
                    )
                    k += 1

            # Scale: u = even+odd partials; s = 1/sqrt(u+eps) computed on the
            # narrow [1, C] vector, THEN broadcast to all partitions with a
            # K=1 matmul into PSUM. Pass-B muls read s straight from PSUM,
            # which keeps the post-broadcast hop off the critical path.
            u_sb = scalep.tile([1, C], mybir.dt.float32)
            upair = u_ps[:, :].rearrange("p (t c) -> p c t", t=2)
            nc.vector.reduce_sum(u_sb, upair, axis=mybir.AxisListType.X)
            tsq = scalep.tile([1, C], mybir.dt.float32)
            nc.scalar.activation(
                out=tsq[:, :],
                in_=u_sb[:, :],
                func=mybir.ActivationFunctionType.Sqrt,
                bias=eps_t[0:1, :],
                scale=1.0,
            )
            s1 = scalep.tile([1, C], mybir.dt.float32)
            nc.vector.reciprocal_approx_fast(out=s1[:, :], in_=tsq[:, :])
            nc.tensor.matmul(
                s_ps[:, 0, :], lhsT=ones_row[:, :], rhs=s1[:, :], start=True, stop=True
            )

            # Pass B: scale cached x, write out. Ramp the group size
            # (2,2,2,2,4,4, then 8s) so the first out-DMA launches right
            # after the scale is ready and the DMA queue never starves
            # while the first full-size mul runs.
            out_groups = (
                [(2 * h, 2) for h in range(4)]
                + [(8, 4), (12, 4)]
                + [(2 * G + j * G, G) for j in range(NG - 2)]
            )
            for t0, g in out_groups:
                ts_ = slice(t0, t0 + g)
                ot = outp.tile([P, g, C], mybir.dt.float32, tag="ot")
                nc.vector.tensor_mul(
                    ot, xc[:, ts_, :], s_ps[:, :, :].to_broadcast((P, g, C))
                )
                nc.sync.dma_start(out=yv[:, ts_, :], in_=ot)
    nc.compile()
    return nc


def _get_nc() -> bass.Bass:
    global _NC
    if _NC is None:
        _NC = _build()
    return _NC


def kernel(x) -> np.ndarray:
    x = np.asarray(x, dtype=np.float32)
    assert x.shape == (N, D), x.shape
    nc = _get_nc()
    in_maps = [
        {"x": np.ascontiguousarray(x[:, i * C : (i + 1) * C])} for i in range(NCORES)
    ]
    try:
        res = run_bass_kernel_spmd(nc, in_maps, core_ids=list(range(NCORES)))
    except Exception:
        # Transient NRT/device hiccups (e.g. a previous process's profiling
        # session left a core wedged) recover after a short pause.
        import time

        time.sleep(5)
        res = run_bass_kernel_spmd(nc, in_maps, core_ids=list(range(NCORES)))
    return np.concatenate([r["y"] for r in res.results], axis=1)
